# revision 1
# baseline (speedup 1.0000x reference)
"""LIF spiking network forward (nn_LIFSG) on 8 Trainium2 NeuronCores.

Math (per reference):
    I = einsum('bti,oi->bto', spikes, W)         # GEMM
    u_t = decay * v_{t-1} + I_t                  # leaky integrate
    s_t = (u_t - 1 > 0)                          # spike
    v_t = u_t * (1 - s_t)                        # reset to zero

Sharding: data-parallel over B (32 batches -> 4 per core). Each core:
  - GEMM as I[o, t] = (W^T tile).T @ (spikes^T tile) on the PE array.
    W is split into 3 bf16 terms (hi/mid/lo); spikes are binary so every
    bf16 product is exact -> fp32-exact GEMM in PSUM. Weight tiles are
    reused across the 4 batches (8 PSUM banks held) to hide LDWEIGHTS.
  - The T=1000 recurrence runs as 1000 fused custom-DVE instructions
    (one LIF step per instruction over all 2048 lanes of the core):
        u_new = select(u_old < nextafter(1), u_old, 0) * decay + I_t
    This is the critical path (~145 ns/dependent step on HW), so time
    chunks are graduated (128/372/500) to start the chain early, and
    spike extraction is moved to the Scalar engine.
  - Spikes extracted on ACT: s = relu(sign(u - 1))  (exact {0,1}),
    streamed out as [b, o, t]; the host transposes back to (B, T, n_out)
    during unshard.

Host-side work is limited to sharding/layout prep (transpose + dtype
cast + W splitting) and the inverse gather; all FLOPs run on device.
"""

import sys

sys.path.insert(0, "/opt/trn_rl_repo")

import numpy as np
import ml_dtypes

import concourse.bacc as bacc
import concourse.tile as tile
import concourse.mybir as mybir
import concourse.dve_ops as dve_ops
from concourse.dve_ops import DveOp
from concourse.dve_spec import C0, C1, Spec, Src0, Src1, Zero, lower, select
from concourse.dve_uop import DveOpSpec
from concourse.bass_utils import run_bass_kernel_spmd

# ---------------- problem constants (hardcoded from spec) ----------------
B, T, N_IN, N_OUT = 32, 1000, 1024, 512
N_CORES = 8
B_SH = B // N_CORES          # 4 batches per core
DECAY = float(np.exp(-1.0 / 20.0))
# u < nextafter(1.0)  <=>  u <= 1.0 in fp32
THRESH_LT = float(np.nextafter(np.float32(1.0), np.float32(np.inf)))

CH_LIST = [128, 372, 500]    # graduated: small head starts the chain early (HW-measured best order)
CH_MAX = max(CH_LIST)
N_IT = N_IN // 128           # 8 contraction tiles
N_OT = N_OUT // 128          # 4 output-partition tiles
LANES = B_SH * N_OT          # 16 scan lanes per core (free dim)
N_SPLIT = 3                  # bf16 splits of W

AF = mybir.ActivationFunctionType


# ---------------- custom DVE op: one LIF step per instruction ----------------
def _lif_ref(in0, in1, c0, c1, c2):
    y = np.where(in0.astype(np.float32) < c1, in0, 0.0).astype(np.float32)
    return (y * np.float32(c0) + in1.astype(np.float32)).astype(np.float32)


_LIF_SPEC = Spec(body=select(Src0 < C1, Src0, Zero) * C0 + Src1, reference=_lif_ref)
_LIF_NAME = "LIF_STEP_ANT"


def _register_lif_op() -> DveOp:
    if _LIF_NAME in dve_ops._SUB_OPCODE_FOR_NAME:
        for op in dve_ops.OPS:
            if op.name == _LIF_NAME:
                return op
    opcode = dve_ops._CUSTOM_DVE_ROW_BASE + len(dve_ops.OPS)
    assert opcode < 0x20
    dve_ops._SUB_OPCODE_FOR_NAME[_LIF_NAME] = opcode
    shas = {}
    for ver in ("v3", "v4"):
        tmp = DveOpSpec(
            name=_LIF_NAME, opcode=opcode, uops=lower(_LIF_SPEC, ver=ver), rd1_en=True
        )
        shas[ver] = tmp.sha(ver)
    op = DveOp(_LIF_NAME, _LIF_SPEC, subdim=False, uops_sha=shas)
    dve_ops.OPS.append(op)
    dve_ops.CUSTOM_DVE_SPECS[_LIF_NAME] = _LIF_SPEC
    return op


# ---------------- device kernel ----------------
def _build_kernel():
    LIF = _register_lif_op()
    nc = bacc.Bacc("TRN2", target_bir_lowering=False, debug=False, num_devices=N_CORES)
    # Register a -1.0 const AP (used as the ACT Sign bias); mirrors the
    # built-in const registration in Bass.__init__.
    _cm1 = nc.alloc_sbuf_tensor("const-float32-neg1", [128, 1], mybir.dt.float32)
    nc.gpsimd.memset(_cm1.ap(), -1.0)
    nc.const_aps.aps[(mybir.dt.float32, -1.0)] = _cm1.ap()
    nc.all_engine_barrier()
    xT = nc.dram_tensor("xT", [B_SH, N_IN, T], mybir.dt.bfloat16, kind="ExternalInput")
    wts = nc.dram_tensor(
        "wts", [N_SPLIT, N_IN, N_OUT], mybir.dt.bfloat16, kind="ExternalInput"
    )
    out = nc.dram_tensor("out", [B_SH, N_OUT, T], mybir.dt.float32, kind="ExternalOutput")

    with tile.TileContext(nc) as tc:
        with (
            tc.tile_pool(name="wx", bufs=1) as wx_pool,
            tc.tile_pool(name="state", bufs=1) as state_pool,
            tc.tile_pool(name="spk", bufs=4) as spk_pool,
            tc.tile_pool(name="mm", bufs=8, space="PSUM") as psum_pool,
        ):
            # Stationary weights: [128p, split, it, o]. One DMA per split so
            # the s=0 matmuls only wait for the first 1MB.
            w_sb = wx_pool.tile([128, N_SPLIT, N_IT, N_OUT], mybir.dt.bfloat16, tag="w")
            wts_r = wts.rearrange("s (it p) o -> p s it o", p=128)
            for s in range(N_SPLIT):
                nc.sync.dma_start(w_sb[:, s], wts_r[:, s])
            # Spike inputs, all resident: one tile per batch [128p, it, T].
            # Split each DMA into head (chunk-0 columns) + rest so the first
            # chunk's matmuls start after ~1MB instead of 8MB of input DMA.
            head = CH_LIST[0]
            x_sb = []
            for b in range(B_SH):
                xt = wx_pool.tile(
                    [128, N_IT, T], mybir.dt.bfloat16, tag=f"x{b}", name=f"x{b}"
                )
                xTb = xT[b].rearrange("(it p) t -> p it t", p=128)
                nc.sync.dma_start(xt[:, :, :head], xTb[:, :, :head])
                nc.sync.dma_start(xt[:, :, head:], xTb[:, :, head:])
                x_sb.append(xt)

            # Scan state (u trajectory) and GEMM output, ping-pong per chunk
            # Per-parity max chunk length (ping-pong buffers sized to need)
            chmax = [max(c for i, c in enumerate(CH_LIST) if i % 2 == k) for k in range(2)]
            U = [
                state_pool.tile(
                    [128, LANES, chmax[k] + 1], mybir.dt.float32, tag=f"U{k}", name=f"U{k}"
                )
                for k in range(2)
            ]
            Ibuf = [
                state_pool.tile(
                    [128, LANES, chmax[k]], mybir.dt.float32, tag=f"I{k}", name=f"I{k}"
                )
                for k in range(2)
            ]
            zero_col = state_pool.tile([128, LANES], mybir.dt.float32, tag="z")
            nc.vector.memset(zero_col[:], 0.0)

            t0 = 0
            prev_ch = 0
            for ic, ch in enumerate(CH_LIST):
                pc = ic % 2
                # ---- GEMM for this chunk: I[o, t] per (b, ot) lane ----
                # Weight tile outer, batch inner: each loaded weight feeds 4
                # matmuls; 8 PSUM banks (2 ot x 4 b) accumulate concurrently.
                for half in range(2):
                    ots = (2 * half, 2 * half + 1)
                    pss = {
                        (ot, b): psum_pool.tile(
                            [128, ch], mybir.dt.float32, tag="ps", name="ps"
                        )
                        for ot in ots
                        for b in range(B_SH)
                    }
                    for s in range(N_SPLIT):
                        for it in range(N_IT):
                            for ot in ots:
                                w_ap = w_sb[:, s, it, ot * 128 : (ot + 1) * 128]
                                for b in range(B_SH):
                                    nc.tensor.matmul(
                                        pss[(ot, b)][:],
                                        w_ap,
                                        x_sb[b][:, it, t0 : t0 + ch],
                                        start=(s == 0 and it == 0),
                                        stop=(s == N_SPLIT - 1 and it == N_IT - 1),
                                    )
                    for ot in ots:
                        for b in range(B_SH):
                            lane = b * N_OT + ot
                            nc.scalar.copy(Ibuf[pc][:, lane, :ch], pss[(ot, b)][:])

                # ---- LIF chain: one custom-DVE instruction per timestep ----
                for j in range(ch):
                    if ic == 0 and j == 0:
                        prev = zero_col[:]
                    elif j == 0:
                        prev = U[1 - pc][:, :, prev_ch]
                    else:
                        prev = U[pc][:, :, j]
                    nc.vector._custom_dve(
                        LIF,
                        out=U[pc][:, :, j + 1],
                        in0=prev,
                        in1=Ibuf[pc][:, :, j],
                        s0=DECAY,
                        s1=THRESH_LT,
                    )

                # ---- spike extraction on ACT: s = relu(sign(u - 1)) ----
                # Last chunk: extract in two column-halves so the first half
                # (and its store) overlaps the still-running chain tail.
                pieces = (
                    [(1, ch // 2), (1 + ch // 2, ch - ch // 2)]
                    if ic == len(CH_LIST) - 1
                    else [(1, ch)]
                )
                for c0, clen in pieces:
                    for b in range(B_SH):
                        for ot in range(N_OT):
                            lane = b * N_OT + ot
                            st = spk_pool.tile(
                                [128, CH_MAX], mybir.dt.float32, tag="s", name="s"
                            )
                            nc.vector.tensor_scalar(
                                st[:, :clen],
                                U[pc][:, lane, c0 : c0 + clen],
                                1.0,
                                None,
                                mybir.AluOpType.is_gt,
                            )
                            nc.sync.dma_start(
                                out[
                                    b,
                                    ot * 128 : (ot + 1) * 128,
                                    t0 + c0 - 1 : t0 + c0 - 1 + clen,
                                ],
                                st[:, :clen],
                            )
                t0 += ch
                prev_ch = ch

    _dedupe_ldweights(nc)
    nc.compile()
    return nc


def _dedupe_ldweights(nc):
    """Remove back-to-back redundant Ldweights.

    The batch-inner GEMM loop issues 4 matmuls per weight tile; bass emits
    an Ldweights per matmul, so 3 of every 4 weight loads re-load the array
    with the bits it already holds (~75us of PE time). The PE keeps the
    stationary operand until the next Ldweights, so a duplicate load whose
    weights AP is identical to the previous one is a no-op -- drop it,
    provided it carries no semaphore waits/updates and only Matmult
    instructions sit in between (nothing else can clobber the array, and
    the weight tile in SBUF is written once at kernel start).
    """

    def _key(inst):
        a = inst.ins[0]
        try:
            return (a.memory_location().name, a.offset, str(a.ap))
        except Exception:
            return None

    removed = 0
    for blk in nc.m.functions[0].blocks:
        prev_key = None
        keep = []
        for inst in blk.instructions:
            if inst.opcode == "Ldweights":
                k = _key(inst)
                plain = not inst.sync_info and k is not None
                if plain and k == prev_key:
                    removed += 1
                    continue
                prev_key = k if plain else None
            elif inst.opcode != "Matmult":
                prev_key = None
            keep.append(inst)
        blk.instructions = keep
    return removed


_NC_CACHE = None


def _prep_inputs(input_spikes_seq: np.ndarray, W: np.ndarray):
    W32 = np.ascontiguousarray(np.asarray(W, dtype=np.float32).T)   # [n_in, n_out]
    w_hi = W32.astype(ml_dtypes.bfloat16)
    r1 = W32 - w_hi.astype(np.float32)
    w_mid = r1.astype(ml_dtypes.bfloat16)
    w_lo = (r1 - w_mid.astype(np.float32)).astype(ml_dtypes.bfloat16)
    wts = np.ascontiguousarray(np.stack([w_hi, w_mid, w_lo])[:N_SPLIT])

    x = np.asarray(input_spikes_seq, dtype=np.float32)
    in_maps = []
    for c in range(N_CORES):
        xs = x[c * B_SH : (c + 1) * B_SH]                           # [4, T, n_in]
        xs_T = np.ascontiguousarray(xs.transpose(0, 2, 1)).astype(ml_dtypes.bfloat16)
        in_maps.append({"xT": xs_T, "wts": wts})
    return in_maps


def kernel(input_spikes_seq: np.ndarray, W: np.ndarray) -> np.ndarray:
    global _NC_CACHE
    if _NC_CACHE is None:
        _NC_CACHE = _build_kernel()
    nc = _NC_CACHE

    in_maps = _prep_inputs(input_spikes_seq, W)
    res = run_bass_kernel_spmd(nc, in_maps, core_ids=list(range(N_CORES)))

    # ---- gather/unshard: [core][4, n_out, T] -> (B, T, n_out) ----
    outs = [r["out"] for r in res.results]
    full = np.concatenate(outs, axis=0)                             # [B, n_out, T]
    return np.ascontiguousarray(full.transpose(0, 2, 1))



# revision 11
# speedup vs baseline: 1.4409x; 1.4409x over previous
"""LIF spiking network forward (nn_LIFSG) on 8 Trainium2 NeuronCores.

Math (per reference):
    I = einsum('bti,oi->bto', spikes, W)         # GEMM
    u_t = decay * v_{t-1} + I_t                  # leaky integrate
    s_t = (u_t - 1 > 0)                          # spike
    v_t = u_t * (1 - s_t)                        # reset to zero

Sharding: data-parallel over B (32 batches -> 4 per core).

GEMM: W is decomposed into 6 fp8-e4m3 "bit slices": W_int = rint(|W|*2^24)
is split into 4-bit fields; slice s holds sign*field*2^(4s-24), stored
scaled by 2^16 (s<3) or 2^8 (s>=3) so every value is exactly representable
in e4m3 (low slice lands on even e4m3-subnormal grid points). The scale is
restored exactly through the moving operand: spikes are sent as e5m2 bytes
valued 2^-16 / 2^-8 (powers of two, exact). All 6 slices accumulate into
one fp32 PSUM group. Within a DoubleRow pair the PE sums products in
~e10m10, but slice values span only 4 bits so pair-sums are exact
(HW-verified). Net weight error ~2^-25 -> bit-accurate spikes.
All matmuls run in DoubleRow fp8 mode (0.5 PE cycles/column).

Recurrence: the T=1000 scan is split into two time segments,
A=[0,644) and B=[356,1000) (outputs from 644; the first 288 steps of B
are warm-up, long enough that the unknown v(356) has decayed below 1e-6).
The two chains are independent, so their per-step custom-DVE instructions
are interleaved on the Vector engine, hiding most of the RAW latency.
I is written by the Activation engine directly into the chain state
buffer (in-place: u_{j+1} overwrites I_{j+1}), spike extraction
(u > 1 -> bf16) runs on the Pool engine, and outputs stream out per chunk.
"""

import sys

sys.path.insert(0, "/opt/trn_rl_repo")

import numpy as np
import ml_dtypes

import concourse.bacc as bacc
import concourse.tile as tile
import concourse.mybir as mybir
import concourse.dve_ops as dve_ops
from concourse.dve_ops import DveOp
from concourse.dve_spec import C0, C1, Spec, Src0, Src1, Zero, lower, select
from concourse.dve_uop import DveOpSpec
from concourse.bass_utils import run_bass_kernel_spmd

# ---------------- problem constants (hardcoded from spec) ----------------
B, T, N_IN, N_OUT = 32, 1000, 1024, 512
N_CORES = 8
B_SH = B // N_CORES          # 4 batches per core
DECAY = float(np.exp(-1.0 / 20.0))
# u < nextafter(1.0)  <=>  u <= 1.0 in fp32
THRESH_LT = float(np.nextafter(np.float32(1.0), np.float32(np.inf)))

# --- time segmentation ---
WARM = 288                   # warm-up steps for segment B (v-error < ~1e-6)
H = (T + WARM) // 2          # 644: segment boundary (B outputs [H, T))
SEG0 = [0, H - WARM]         # global start of each segment's chain
SEG_LEN = H                  # both chains run 644 steps
CHUNKS = [64, 128, 196, 256] # per-segment chunk sizes (sum = 644)
assert sum(CHUNKS) == SEG_LEN
CHMAX = [max(c for i, c in enumerate(CHUNKS) if i % 2 == p) for p in (0, 1)]

# --- fp8 weight slicing ---
SHIFT = 24
N_SL = 6
SL_SC = [16, 16, 16, 8, 8, 8]          # stored-value scale exponent per slice
SL_SLAB = [1, 1, 1, 0, 0, 0]           # x slab index per slice (0:2^-8, 1:2^-16)
XB_HI, XB_LO = 0x1C, 0x01              # e5m2 bytes for 2^-8 and 2^-16

N_IT = N_IN // 128           # 8 contraction tiles
N_OT = N_OUT // 128          # 4 output-partition tiles
LANES = B_SH * N_OT          # 16 scan lanes per core
E4 = ml_dtypes.float8_e4m3
E5 = ml_dtypes.float8_e5m2


# ---------------- custom DVE op: one LIF step per instruction ----------------
def _lif_ref(in0, in1, c0, c1, c2):
    y = np.where(in0.astype(np.float32) < c1, in0, 0.0).astype(np.float32)
    return (y * np.float32(c0) + in1.astype(np.float32)).astype(np.float32)


_LIF_SPEC = Spec(body=select(Src0 < C1, Src0, Zero) * C0 + Src1, reference=_lif_ref)
_LIF_NAME = "LIF_STEP_ANT"


def _register_lif_op() -> DveOp:
    if _LIF_NAME in dve_ops._SUB_OPCODE_FOR_NAME:
        for op in dve_ops.OPS:
            if op.name == _LIF_NAME:
                return op
    opcode = dve_ops._CUSTOM_DVE_ROW_BASE + len(dve_ops.OPS)
    assert opcode < 0x20
    dve_ops._SUB_OPCODE_FOR_NAME[_LIF_NAME] = opcode
    shas = {}
    for ver in ("v3", "v4"):
        tmp = DveOpSpec(
            name=_LIF_NAME, opcode=opcode, uops=lower(_LIF_SPEC, ver=ver), rd1_en=True
        )
        shas[ver] = tmp.sha(ver)
    op = DveOp(_LIF_NAME, _LIF_SPEC, subdim=False, uops_sha=shas)
    dve_ops.OPS.append(op)
    dve_ops.CUSTOM_DVE_SPECS[_LIF_NAME] = _LIF_SPEC
    return op


# ---------------- device kernel ----------------
def _build_kernel():
    LIF = _register_lif_op()
    nc = bacc.Bacc("TRN2", target_bir_lowering=False, debug=False, num_devices=N_CORES)
    wts = nc.dram_tensor(
        "wts", [128, N_SL, N_IT, N_OUT], mybir.dt.float8e4, kind="ExternalInput"
    )
    xts = {}
    for si in range(2):
        for ci, ch in enumerate(CHUNKS):
            xts[(si, ci)] = nc.dram_tensor(
                f"x_{si}_{ci}", [128, B_SH, 2, N_IT, ch], mybir.dt.float8e5,
                kind="ExternalInput",
            )
    out = nc.dram_tensor("out", [B_SH, N_OUT, T], mybir.dt.bfloat16, kind="ExternalOutput")

    with tile.TileContext(nc) as tc:
        with (
            tc.tile_pool(name="w", bufs=1) as w_pool,
            tc.tile_pool(name="x", bufs=1) as x_pool,
            tc.tile_pool(name="state", bufs=1) as state_pool,
            tc.tile_pool(name="spk", bufs=2) as spk_pool,
            tc.tile_pool(name="mm", bufs=8, space="PSUM") as psum_pool,
        ):
            # Stationary weight slices: [128p, slice, it, o]; one contiguous DMA.
            w_sb = w_pool.tile([128, N_SL, N_IT, N_OUT], mybir.dt.float8e4, tag="w")
            nc.sync.dma_start(w_sb[:], wts[:])

            # Per-(segment, chunk) x tiles: [128p, b, slab, it, ch], exact size.
            # All DMAs are issued up front in consumption order; the queue
            # drains ahead of the PE.
            x_sb = {
                (si, ci): x_pool.tile(
                    [128, B_SH, 2, N_IT, ch], mybir.dt.float8e5,
                    tag=f"x{si}{ci}", name=f"x{si}{ci}",
                )
                for si in range(2) for ci, ch in enumerate(CHUNKS)
            }
            # Chain state (u trajectory, I written in place): [128p, lane, ch+1]
            U = {
                (si, par): state_pool.tile(
                    [128, LANES, CHMAX[par] + 1], mybir.dt.float32,
                    tag=f"U{si}{par}", name=f"U{si}{par}",
                )
                for si in range(2) for par in range(2)
            }
            zero_col = state_pool.tile([128, LANES], mybir.dt.float32, tag="z")
            nc.vector.memset(zero_col[:], 0.0)

            for ci in range(len(CHUNKS)):
                for si in range(2):
                    nc.sync.dma_start(x_sb[(si, ci)][:], xts[(si, ci)][:])

            t0 = 0
            prev_ch = 0
            for ci, ch in enumerate(CHUNKS):
                par = ci % 2

                # ---- GEMM both segments' chunks; quarters of 4 batches ----
                for si in range(2):
                    xs = x_sb[(si, ci)]
                    for ot in range(N_OT):
                        pss = [
                            psum_pool.tile([128, 512], mybir.dt.float32, tag="ps", name="ps")
                            for _ in range(B_SH)
                        ]
                        for s in range(N_SL):
                            sl = SL_SLAB[s]
                            for kp in range(N_IT // 2):
                                w_ap = w_sb[:, s, 2 * kp : 2 * kp + 2,
                                            ot * 128 : (ot + 1) * 128]
                                for b in range(B_SH):
                                    nc.tensor.matmul(
                                        pss[b][:, :ch],
                                        w_ap,
                                        xs[:, b, sl, 2 * kp : 2 * kp + 2, :],
                                        start=(s == 0 and kp == 0),
                                        stop=(s == N_SL - 1 and kp == N_IT // 2 - 1),
                                        perf_mode=mybir.MatmulPerfMode.DoubleRow,
                                    )
                        # drain quarter: I -> U columns [1..ch] (in place)
                        for b in range(B_SH):
                            lane = b * N_OT + ot
                            nc.scalar.copy(
                                U[(si, par)][:, lane, 1 : 1 + ch], pss[b][:, :ch]
                            )

                # ---- interleaved LIF chains: A step j, B step j ----
                for j in range(ch):
                    for si in range(2):
                        u = U[(si, par)]
                        if ci == 0 and j == 0:
                            prev = zero_col[:]
                        elif j == 0:
                            prev = U[(si, 1 - par)][:, :, prev_ch]
                        else:
                            prev = u[:, :, j]
                        nc.vector._custom_dve(
                            LIF, out=u[:, :, j + 1], in0=prev, in1=u[:, :, j + 1],
                            s0=DECAY, s1=THRESH_LT,
                        )

                # ---- spike extraction on Pool + stream out ----
                for si in range(2):
                    gt0 = SEG0[si] + t0           # global t of chunk start
                    e0 = 0 if si == 0 else max(H - gt0, 0)  # B outputs from H
                    elen = ch - e0
                    if elen <= 0:
                        continue
                    st = spk_pool.tile(
                        [128, LANES, CHMAX[1]], mybir.dt.bfloat16, tag="s", name="s"
                    )
                    # last chunk: two column-halves so extraction overlaps the
                    # still-running chain tail of the other segment
                    pieces = (
                        [(e0, elen // 2), (e0 + elen // 2, elen - elen // 2)]
                        if ci == len(CHUNKS) - 1
                        else [(e0, elen)]
                    )
                    for p0, plen in pieces:
                        if plen <= 0:
                            continue
                        nc.gpsimd.tensor_scalar(
                            st[:, :, p0 : p0 + plen],
                            U[(si, par)][:, :, 1 + p0 : 1 + p0 + plen],
                            1.0, None, mybir.AluOpType.is_gt,
                        )
                        for b in range(B_SH):
                            for ot in range(N_OT):
                                lane = b * N_OT + ot
                                nc.sync.dma_start(
                                    out[b, ot * 128 : (ot + 1) * 128,
                                        gt0 + p0 : gt0 + p0 + plen],
                                    st[:, lane, p0 : p0 + plen],
                                )
                t0 += ch
                prev_ch = ch

    _dedupe_ldweights(nc)
    nc.compile()
    return nc


def _dedupe_ldweights(nc):
    """Remove back-to-back redundant Ldweights (batch-inner matmul loops load
    the same stationary tile 4x; the PE keeps it until the next Ldweights)."""

    def _key(inst):
        a = inst.ins[0]
        try:
            return (a.memory_location().name, a.offset, str(a.ap))
        except Exception:
            return None

    removed = 0
    for blk in nc.m.functions[0].blocks:
        prev_key = None
        keep = []
        for inst in blk.instructions:
            if inst.opcode == "Ldweights":
                k = _key(inst)
                plain = not inst.sync_info and k is not None
                if plain and k == prev_key:
                    removed += 1
                    continue
                prev_key = k if plain else None
            elif inst.opcode != "Matmult":
                prev_key = None
            keep.append(inst)
        blk.instructions = keep
    return removed


_NC_CACHE = None


def _slice_weights(W: np.ndarray) -> np.ndarray:
    """[n_out, n_in] fp32 -> [128, N_SL, N_IT, N_OUT] e4m3 slice tensor."""
    W64 = W.astype(np.float64).T                      # [n_in, n_out]
    Wi = np.rint(np.abs(W64) * 2.0 ** SHIFT)
    assert Wi.max() < 16.0 ** N_SL
    sg = np.sign(W64)
    slices = []
    for s in range(N_SL):
        field = np.floor(Wi / 16.0 ** s) % 16
        stored = sg * field * (2.0 ** (4 * s - SHIFT + SL_SC[s]))
        st8 = stored.astype(E4)
        assert np.array_equal(st8.astype(np.float64), stored), f"slice {s} inexact"
        slices.append(st8)
    arr = np.stack(slices)                            # [S, n_in, n_out]
    arr = arr.reshape(N_SL, N_IT, 128, N_OUT).transpose(2, 0, 1, 3)
    return np.ascontiguousarray(arr)                  # [128, S, it, o]


def _prep_inputs(input_spikes_seq: np.ndarray, W: np.ndarray):
    w_sl = _slice_weights(np.asarray(W, dtype=np.float32))

    mask = (np.asarray(input_spikes_seq) != 0).astype(np.uint8)   # [B, T, n_in]
    in_maps = []
    for c in range(N_CORES):
        m = mask[c * B_SH : (c + 1) * B_SH]                       # [4, T, n_in]
        # -> [128p, b, it, T]
        base = np.ascontiguousarray(
            m.transpose(0, 2, 1).reshape(B_SH, N_IT, 128, T).transpose(2, 0, 1, 3)
        )
        im = {"wts": w_sl}
        for si in range(2):
            t0 = SEG0[si]
            for ci, ch in enumerate(CHUNKS):
                sl = base[:, :, :, t0 : t0 + ch]                  # [128, b, it, ch]
                arr = np.empty((128, B_SH, 2, N_IT, ch), dtype=np.uint8)
                arr[:, :, 0] = sl * XB_HI
                arr[:, :, 1] = sl * XB_LO
                im[f"x_{si}_{ci}"] = arr.view(E5)
                t0 += ch
        in_maps.append(im)
    return in_maps


def kernel(input_spikes_seq: np.ndarray, W: np.ndarray) -> np.ndarray:
    global _NC_CACHE
    if _NC_CACHE is None:
        _NC_CACHE = _build_kernel()
    nc = _NC_CACHE

    in_maps = _prep_inputs(input_spikes_seq, W)
    res = run_bass_kernel_spmd(nc, in_maps, core_ids=list(range(N_CORES)))

    # ---- gather/unshard: [core][4, n_out, T] bf16 -> (B, T, n_out) fp32 ----
    outs = [np.asarray(r["out"]) for r in res.results]
    full = np.concatenate(outs, axis=0)                           # [B, n_out, T]
    return np.ascontiguousarray(full.transpose(0, 2, 1)).astype(np.float32)
